# revision 8
# baseline (speedup 1.0000x reference)
"""GIN-style GNN (2 layers) on 8 NeuronCores — single launch, collectives,
hardware For_i loops (small NEFF => fast in-launch compile).

Each core owns N/8 = 6250 nodes (49 tiles of 128, last 106). One NEFF:
  h0 embed own slice (16-entry combined table) -> AllGather h0
  per layer: edge gather + one-hot segment-sum matmuls + MLP (For_i over
  48 full tiles + unrolled 106-col tail) -> AllReduce BN stats -> BN apply
  (+relu for l=0) -> transpose -> AllGather h1 / bf16 output slice.
Host does integer prep: bucket+sort edges by dst, pad to uniform K chunks
of 128 edges per tile, 21-class per-node histogram; indices ship as
u16/u8 and widen on device.
"""

import sys

sys.path.insert(0, "/opt/trn_rl_repo")

import numpy as np

import concourse.bass as bass
import concourse.tile as tile
from concourse import bacc, mybir
from concourse.bass_utils import run_bass_kernel_spmd

N = 50000
E = 800000
D = 128
P = 128
NCORES = 8
NPC = N // NCORES          # 6250 nodes per core
NT = (NPC + P - 1) // P    # 49 tiles; last has 106 rows
NFULL = NT - 1             # 48 full tiles
TAILC = NPC - NFULL * P    # 106
BN_EPS = 1e-5
F32 = mybir.dt.float32
I32 = mybir.dt.int32
U16 = mybir.dt.uint16
U8 = mybir.dt.uint8
BF16 = mybir.dt.bfloat16


def _pack_cols(arr1d):
    n = arr1d.shape[0]
    return np.ascontiguousarray(arr1d.reshape(n // P, P).T)


def _host_prep(x, edge_index, edge_attr):
    ei = np.asarray(edge_index)
    ea = np.asarray(edge_attr)

    loop = np.arange(N, dtype=np.int64)
    src = np.concatenate([ei[0], loop]).astype(np.int64)
    dst = np.concatenate([ei[1], loop]).astype(np.int64)
    t = np.concatenate([ea[:, 0] * 3 + ea[:, 1], np.full(N, 4 * 3, np.int64)])

    per_core = []
    counts_all = []
    for c in range(NCORES):
        lo, hi = c * NPC, (c + 1) * NPC
        m = (dst >= lo) & (dst < hi)
        es, ed, et = src[m], dst[m] - lo, t[m]
        order = np.argsort(ed, kind="stable")
        es, ed, et = es[order], ed[order], et[order]
        bounds = np.searchsorted(ed, np.arange(0, NPC + P, P))
        per_core.append((es, ed, et, bounds))
        counts_all.append(bounds[1:NT + 1] - bounds[:NT])
    K = int(np.max([np.ceil(c / P) for c in np.concatenate(counts_all)]))

    packed = []
    for c in range(NCORES):
        es, ed, et, bounds = per_core[c]
        srcg = np.zeros((NT, K * P), np.uint16)
        dstg = np.full((NT, K * P), 255, np.uint8)
        cntT = np.zeros((NPC, 21), np.int64)
        np.add.at(cntT, (ed, et), 1)
        assert cntT.max() < 256 and es.max() < 65536
        for ti in range(NT):
            a, b = bounds[ti], bounds[ti + 1]
            n = b - a
            srcg[ti, :n] = es[a:b]
            dstg[ti, :n] = (ed[a:b] - ti * P).astype(np.uint8)
        packed.append({
            "srcp": _pack_cols(srcg.reshape(-1)),          # [128, NT*K] u16
            "dstp": _pack_cols(dstg.reshape(-1)),          # [128, NT*K] u8
            "cntT": np.ascontiguousarray(cntT.T.astype(np.uint8)),  # [21, NPC]
        })
    return packed, K


def _load_const(nc, pool, dram_ap, shape, dtype):
    sb = pool.tile(shape, dtype, name=f"c_{dram_ap.name}")
    nc.sync.dma_start(out=sb[:], in_=dram_ap[:])
    return sb


def _build(K, TAB):
    nc = bacc.Bacc(None, target_bir_lowering=False)
    RG = [list(range(NCORES))]

    xi = nc.dram_tensor("xi", [P, NT], U16, kind="ExternalInput")
    comb = nc.dram_tensor("comb", [TAB, D], F32, kind="ExternalInput")
    srcp = nc.dram_tensor("srcp", [P, NT * K], U16, kind="ExternalInput")
    dstp = nc.dram_tensor("dstp", [P, NT * K], U8, kind="ExternalInput")
    cntT = nc.dram_tensor("cntT", [21, NPC], U8, kind="ExternalInput")
    etab = [nc.dram_tensor(f"etab{l}", [21, D], F32, kind="ExternalInput")
            for l in range(2)]
    w1 = [nc.dram_tensor(f"w1_{l}", [D, 2 * D], F32, kind="ExternalInput")
          for l in range(2)]
    w2a = [nc.dram_tensor(f"w2a_{l}", [D, D], F32, kind="ExternalInput")
           for l in range(2)]
    w2b = [nc.dram_tensor(f"w2b_{l}", [D, D], F32, kind="ExternalInput")
           for l in range(2)]
    b1a = [nc.dram_tensor(f"b1a_{l}", [D, 1], F32, kind="ExternalInput")
           for l in range(2)]
    b1b = [nc.dram_tensor(f"b1b_{l}", [D, 1], F32, kind="ExternalInput")
           for l in range(2)]
    b2 = [nc.dram_tensor(f"b2_{l}", [D, 1], F32, kind="ExternalInput")
          for l in range(2)]
    gam = [nc.dram_tensor(f"gam{l}", [D, 1], F32, kind="ExternalInput")
           for l in range(2)]
    bet = [nc.dram_tensor(f"bet{l}", [D, 1], F32, kind="ExternalInput")
           for l in range(2)]
    outr = nc.dram_tensor("outr", [NPC, D], BF16, kind="ExternalOutput")

    from contextlib import ExitStack
    with tile.TileContext(nc) as tc, ExitStack() as ctx:
        const = ctx.enter_context(tc.tile_pool(name="const", bufs=1))
        work = ctx.enter_context(tc.tile_pool(name="work", bufs=4))
        psA = ctx.enter_context(tc.tile_pool(name="psA", bufs=1, space="PSUM"))
        psB = ctx.enter_context(tc.tile_pool(name="psB", bufs=1, space="PSUM"))
        psC = ctx.enter_context(tc.tile_pool(name="psC", bufs=1, space="PSUM"))
        dram = ctx.enter_context(tc.tile_pool(name="dram", bufs=1,
                                              space="DRAM"))

        hloc = [dram.tile([NPC, D], F32, name=f"h{l}loc") for l in range(2)]
        hfull = [dram.tile([N, D], F32, name=f"h{l}full") for l in range(2)]
        st_in = [dram.tile([P, 2], F32, name=f"st_in{l}") for l in range(2)]
        st_out = [dram.tile([P, 2], F32, name=f"st_out{l}") for l in range(2)]

        xi_u = _load_const(nc, const, xi, [P, NT], U16)
        xi_sb = const.tile([P, NT], I32)
        nc.vector.tensor_copy(out=xi_sb[:], in_=xi_u[:])
        srcp_u = _load_const(nc, const, srcp, [P, NT * K], U16)
        srcp_sb = const.tile([P, NT * K], I32)
        nc.vector.tensor_copy(out=srcp_sb[:], in_=srcp_u[:])
        dstp_u = _load_const(nc, const, dstp, [P, NT * K], U8)
        dstp_sb = const.tile([P, NT * K], F32)
        nc.vector.tensor_copy(out=dstp_sb[:], in_=dstp_u[:])
        cnt_u = _load_const(nc, const, cntT, [21, NPC], U8)
        cnt_sb = const.tile([21, NPC], F32)
        nc.vector.tensor_copy(out=cnt_sb[:], in_=cnt_u[:])
        iota_i = const.tile([P, P], I32)
        nc.gpsimd.iota(iota_i[:], [[1, P]], channel_multiplier=0)
        iota_sb = const.tile([P, P], F32)
        nc.vector.tensor_copy(out=iota_sb[:], in_=iota_i[:])

        etab_sb = [_load_const(nc, const, etab[l], [21, D], F32)
                   for l in range(2)]
        w1_sb = [_load_const(nc, const, w1[l], [D, 2 * D], F32)
                 for l in range(2)]
        w2a_sb = [_load_const(nc, const, w2a[l], [D, D], F32)
                  for l in range(2)]
        w2b_sb = [_load_const(nc, const, w2b[l], [D, D], F32)
                  for l in range(2)]
        b1a_sb = [_load_const(nc, const, b1a[l], [D, 1], F32)
                  for l in range(2)]
        b1b_sb = [_load_const(nc, const, b1b[l], [D, 1], F32)
                  for l in range(2)]
        b2_sb = [_load_const(nc, const, b2[l], [D, 1], F32) for l in range(2)]
        gam_sb = [_load_const(nc, const, gam[l], [D, 1], F32)
                  for l in range(2)]
        bet_sb = [_load_const(nc, const, bet[l], [D, 1], F32)
                  for l in range(2)]
        ident = const.tile([P, P], F32)
        from concourse.masks import make_identity
        make_identity(nc, ident[:])

        h2all = const.tile([P, NPC], F32)  # reused across both layers

        # ---- h0 for own slice ----
        # indirect-DMA offsets must be static APs: stage the current
        # column into a fixed tile, then gather from it.
        xi_cur = const.tile([P, 1], I32, name="xi_cur")

        def h0_body(ti, rows):
            nc.vector.tensor_copy(out=xi_cur[:], in_=xi_sb[:, bass.ds(ti, 1)])
            hg = work.tile([P, D], F32, name="h0hg")
            nc.gpsimd.indirect_dma_start(
                out=hg[:], out_offset=None, in_=comb[:],
                in_offset=bass.IndirectOffsetOnAxis(ap=xi_cur[:], axis=0))
            nc.sync.dma_start(out=hloc[0][bass.ds(ti * P, rows), :],
                              in_=hg[:rows, :])

        with tc.For_i(0, NFULL) as ti:
            h0_body(ti, P)
        h0_body(NFULL, TAILC)
        nc.gpsimd.collective_compute(
            "AllGather", mybir.AluOpType.bypass, replica_groups=RG,
            ins=[hloc[0].opt()], outs=[hfull[0].opt()])

        src_cur = [const.tile([P, 1], I32, name=f"src_cur{j}")
                   for j in range(K)]

        for l in range(2):
            s_acc = const.tile([P, 2], F32, name=f"s_acc{l}")
            nc.vector.memset(s_acc[:], 0.0)

            def agg_body(ti, cols, l=l):
                agg_ps = psA.tile([P, P], F32, space="PSUM", name="agg_ps")
                nc.tensor.matmul(
                    out=agg_ps[:, :cols], lhsT=etab_sb[l][:],
                    rhs=cnt_sb[:, bass.ds(ti * P, cols)],
                    start=True, stop=False, skip_group_check=True)
                for j in range(K):
                    col = ti * K + j
                    nc.vector.tensor_copy(out=src_cur[j][:],
                                          in_=srcp_sb[:, bass.ds(col, 1)])
                    hg = work.tile([P, D], F32, name="ehg")
                    nc.gpsimd.indirect_dma_start(
                        out=hg[:], out_offset=None, in_=hfull[l][:],
                        in_offset=bass.IndirectOffsetOnAxis(
                            ap=src_cur[j][:], axis=0))
                    oh = work.tile([P, P], F32, name="eoh")
                    nc.vector.tensor_tensor(
                        out=oh[:, :cols],
                        in0=dstp_sb[:, bass.ds(col, 1)].to_broadcast([P, cols]),
                        in1=iota_sb[:, :cols], op=mybir.AluOpType.is_equal)
                    nc.tensor.matmul(
                        out=agg_ps[:, :cols], lhsT=hg[:], rhs=oh[:, :cols],
                        start=False, stop=(j == K - 1), skip_group_check=True)
                aggT = work.tile([P, P], F32, name="aggT")
                nc.vector.tensor_copy(out=aggT[:, :cols], in_=agg_ps[:, :cols])

                r = []
                for half, bsb in ((0, b1a_sb[l]), (1, b1b_sb[l])):
                    z_ps = psB.tile([P, P], F32, space="PSUM", name=f"z{half}")
                    nc.tensor.matmul(
                        out=z_ps[:, :cols],
                        lhsT=w1_sb[l][:, half * D:(half + 1) * D],
                        rhs=aggT[:, :cols], start=True, stop=True,
                        skip_group_check=True)
                    rh = work.tile([P, P], F32, name=f"rh{half}")
                    nc.vector.tensor_tensor(
                        out=rh[:, :cols], in0=z_ps[:, :cols],
                        in1=bsb[:, :1].to_broadcast([P, cols]),
                        op=mybir.AluOpType.add)
                    nc.vector.tensor_scalar_max(rh[:, :cols], rh[:, :cols],
                                                0.0)
                    r.append(rh)
                h2_ps = psC.tile([P, P], F32, space="PSUM", name="h2ps")
                nc.tensor.matmul(out=h2_ps[:, :cols], lhsT=w2a_sb[l][:],
                                 rhs=r[0][:, :cols], start=True, stop=False,
                                 skip_group_check=True)
                nc.tensor.matmul(out=h2_ps[:, :cols], lhsT=w2b_sb[l][:],
                                 rhs=r[1][:, :cols], start=False, stop=True,
                                 skip_group_check=True)
                dst = h2all[:, bass.ds(ti * P, cols)]
                nc.vector.tensor_tensor(
                    out=dst, in0=h2_ps[:, :cols],
                    in1=b2_sb[l][:, :1].to_broadcast([P, cols]),
                    op=mybir.AluOpType.add)
                part = work.tile([P, 1], F32, name="part")
                nc.vector.reduce_sum(out=part[:], in_=dst,
                                     axis=mybir.AxisListType.X)
                nc.vector.tensor_add(s_acc[:, 0:1], s_acc[:, 0:1], part[:])
                sq = work.tile([P, P], F32, name="sq")
                nc.vector.tensor_mul(sq[:, :cols], dst, dst)
                part2 = work.tile([P, 1], F32, name="part2")
                nc.vector.reduce_sum(out=part2[:], in_=sq[:, :cols],
                                     axis=mybir.AxisListType.X)
                nc.vector.tensor_add(s_acc[:, 1:2], s_acc[:, 1:2], part2[:])

            with tc.For_i(0, NFULL) as ti:
                agg_body(ti, P)
            agg_body(NFULL, TAILC)

            # ---- AllReduce BN stats ----
            nc.sync.dma_start(out=st_in[l][:], in_=s_acc[:])
            nc.gpsimd.collective_compute(
                "AllReduce", mybir.AluOpType.add, replica_groups=RG,
                ins=[st_in[l].opt()], outs=[st_out[l].opt()])
            st_sb = const.tile([P, 2], F32, name=f"st_sb{l}")
            nc.sync.dma_start(out=st_sb[:], in_=st_out[l][:])

            # BN coefficients: a = gamma*rsqrt(var+eps), b = beta - a*mu
            mu = const.tile([P, 1], F32, name=f"mu{l}")
            nc.vector.tensor_scalar_mul(mu[:], st_sb[:, 0:1], 1.0 / N)
            ex2 = const.tile([P, 1], F32, name=f"ex2{l}")
            nc.vector.tensor_scalar_mul(ex2[:], st_sb[:, 1:2], 1.0 / N)
            var = const.tile([P, 1], F32, name=f"var{l}")
            nc.vector.tensor_mul(var[:], mu[:], mu[:])
            nc.vector.tensor_tensor(out=var[:], in0=ex2[:], in1=var[:],
                                    op=mybir.AluOpType.subtract)
            nc.vector.tensor_scalar_add(var[:], var[:], BN_EPS)
            std = const.tile([P, 1], F32, name=f"std{l}")
            nc.scalar.activation(out=std[:], in_=var[:],
                                 func=mybir.ActivationFunctionType.Sqrt)
            rstd = const.tile([P, 1], F32, name=f"rstd{l}")
            nc.vector.reciprocal(out=rstd[:], in_=std[:])
            a = const.tile([P, 1], F32, name=f"a{l}")
            nc.vector.tensor_mul(a[:], gam_sb[l][:], rstd[:])
            b = const.tile([P, 1], F32, name=f"b{l}")
            nc.vector.tensor_mul(b[:], a[:], mu[:])
            nc.vector.tensor_tensor(out=b[:], in0=bet_sb[l][:], in1=b[:],
                                    op=mybir.AluOpType.subtract)

            # ---- BN apply (+relu l=0), transpose, write rows ----
            def bn_body(ti, cols, l=l, a=a, b=b):
                src = h2all[:, bass.ds(ti * P, cols)]
                xt = work.tile([P, P], F32, name="bnxt")
                nc.vector.tensor_tensor(
                    out=xt[:, :cols], in0=src,
                    in1=a[:, :1].to_broadcast([P, cols]),
                    op=mybir.AluOpType.mult)
                nc.vector.tensor_tensor(
                    out=xt[:, :cols], in0=xt[:, :cols],
                    in1=b[:, :1].to_broadcast([P, cols]),
                    op=mybir.AluOpType.add)
                if l == 0:
                    nc.vector.tensor_scalar_max(xt[:, :cols], xt[:, :cols],
                                                0.0)
                tp = psC.tile([P, P], F32, space="PSUM", name="bntp")
                nc.tensor.transpose(out=tp[:cols, :], in_=xt[:, :cols],
                                    identity=ident[:])
                if l == 0:
                    hrow = work.tile([P, D], F32, name="bnrow0")
                    nc.vector.tensor_copy(out=hrow[:cols, :], in_=tp[:cols, :])
                    nc.sync.dma_start(out=hloc[1][bass.ds(ti * P, cols), :],
                                      in_=hrow[:cols, :])
                else:
                    hrow = work.tile([P, D], BF16, name="bnrow1")
                    nc.vector.tensor_copy(out=hrow[:cols, :], in_=tp[:cols, :])
                    nc.sync.dma_start(out=outr[bass.ds(ti * P, cols), :],
                                      in_=hrow[:cols, :])

            with tc.For_i(0, NFULL) as ti:
                bn_body(ti, P)
            bn_body(NFULL, TAILC)

            if l == 0:
                nc.gpsimd.collective_compute(
                    "AllGather", mybir.AluOpType.bypass, replica_groups=RG,
                    ins=[hloc[1].opt()], outs=[hfull[1].opt()])
    nc.compile()
    return nc


LAUNCH_NS = []


def _run(nc, maps, cores):
    import time as _t
    t0 = _t.monotonic_ns()
    res = run_bass_kernel_spmd(nc, maps, cores)
    dt = _t.monotonic_ns() - t0
    LAUNCH_NS.append(res.exec_time_ns if res.exec_time_ns else dt)
    return res


def kernel(x, edge_index, edge_attr, batch, xemb1, xemb2, e1, e2,
           W1, b1, W2, b2, gamma, beta):
    LAUNCH_NS.clear()
    f32 = np.float32
    packed, K = _host_prep(x, edge_index, edge_attr)

    x = np.asarray(x)
    nu0 = int(x[:, 0].max()) + 1
    nu1 = int(x[:, 1].max()) + 1
    xi_full = (x[:, 0] * nu1 + x[:, 1]).astype(np.uint16)
    TAB = max(16, nu0 * nu1)
    comb = np.zeros((TAB, D), f32)
    comb[:nu0 * nu1] = (np.asarray(xemb1, f32)[:nu0, None, :]
                        + np.asarray(xemb2, f32)[None, :nu1, :]
                        ).reshape(nu0 * nu1, D)

    shared = {"comb": comb}
    for l in range(2):
        shared[f"etab{l}"] = (
            np.repeat(np.asarray(e1[l], f32), 3, axis=0)
            + np.tile(np.asarray(e2[l], f32), (7, 1))).copy()
        shared[f"w1_{l}"] = np.asarray(W1[l], f32).copy()
        shared[f"w2a_{l}"] = np.asarray(W2[l][:D], f32).copy()
        shared[f"w2b_{l}"] = np.asarray(W2[l][D:], f32).copy()
        shared[f"b1a_{l}"] = np.asarray(b1[l][:D], f32).reshape(D, 1).copy()
        shared[f"b1b_{l}"] = np.asarray(b1[l][D:], f32).reshape(D, 1).copy()
        shared[f"b2_{l}"] = np.asarray(b2[l], f32).reshape(D, 1).copy()
        shared[f"gam{l}"] = np.asarray(gamma[l], f32).reshape(D, 1).copy()
        shared[f"bet{l}"] = np.asarray(beta[l], f32).reshape(D, 1).copy()

    cores = list(range(NCORES))
    nc = _build(K, TAB)
    maps = []
    for c in cores:
        xi_pad = np.zeros(NT * P, np.uint16)
        xi_pad[:NPC] = xi_full[c * NPC:(c + 1) * NPC]
        m = {"xi": _pack_cols(xi_pad),
             "srcp": packed[c]["srcp"], "dstp": packed[c]["dstp"],
             "cntT": packed[c]["cntT"]}
        m.update(shared)
        maps.append(m)
    res = _run(nc, maps, cores).results
    out = np.concatenate([r["outr"] for r in res], axis=0)
    return out.astype(np.float32)


# revision 9
# speedup vs baseline: 1.2651x; 1.2651x over previous
"""GIN-style GNN (2 layers) on 8 NeuronCores — single launch, collectives,
hardware For_i loops (small NEFF => fast in-launch compile).

Each core owns N/8 = 6250 nodes (49 tiles of 128, last 106). One NEFF:
  h0 embed own slice (16-entry combined table) -> AllGather h0
  per layer: edge gather + one-hot segment-sum matmuls + MLP (For_i over
  48 full tiles + unrolled 106-col tail) -> AllReduce BN stats -> BN apply
  (+relu for l=0) -> transpose -> AllGather h1 / bf16 output slice.
Host does integer prep: bucket+sort edges by dst, pad to uniform K chunks
of 128 edges per tile, 21-class per-node histogram; indices ship as
u16/u8 and widen on device.
"""

import sys

sys.path.insert(0, "/opt/trn_rl_repo")

import numpy as np

import concourse.bass as bass
import concourse.tile as tile
from concourse import bacc, mybir
from concourse.bass_utils import run_bass_kernel_spmd

N = 50000
E = 800000
D = 128
P = 128
NCORES = 8
NPC = N // NCORES          # 6250 nodes per core
NT = (NPC + P - 1) // P    # 49 tiles; last has 106 rows
NFULL = NT - 1             # 48 full tiles
TAILC = NPC - NFULL * P    # 106
BN_EPS = 1e-5
F32 = mybir.dt.float32
I32 = mybir.dt.int32
U16 = mybir.dt.uint16
U8 = mybir.dt.uint8
BF16 = mybir.dt.bfloat16


def _pack_cols(arr1d):
    n = arr1d.shape[0]
    return np.ascontiguousarray(arr1d.reshape(n // P, P).T)


def _host_prep(x, edge_index, edge_attr):
    ei = np.asarray(edge_index)
    ea = np.asarray(edge_attr)

    loop = np.arange(N, dtype=np.int64)
    src = np.concatenate([ei[0], loop]).astype(np.int64)
    dst = np.concatenate([ei[1], loop]).astype(np.int64)
    t = np.concatenate([ea[:, 0] * 3 + ea[:, 1], np.full(N, 4 * 3, np.int64)])

    per_core = []
    counts_all = []
    for c in range(NCORES):
        lo, hi = c * NPC, (c + 1) * NPC
        m = (dst >= lo) & (dst < hi)
        es, ed, et = src[m], dst[m] - lo, t[m]
        order = np.argsort(ed, kind="stable")
        es, ed, et = es[order], ed[order], et[order]
        bounds = np.searchsorted(ed, np.arange(0, NPC + P, P))
        per_core.append((es, ed, et, bounds))
        counts_all.append(bounds[1:NT + 1] - bounds[:NT])
    K = int(np.max([np.ceil(c / P) for c in np.concatenate(counts_all)]))

    packed = []
    for c in range(NCORES):
        es, ed, et, bounds = per_core[c]
        srcg = np.zeros((NT, K * P), np.uint16)
        dstg = np.full((NT, K * P), 255, np.uint8)
        cntT = np.zeros((NPC, 21), np.int64)
        np.add.at(cntT, (ed, et), 1)
        assert cntT.max() < 256 and es.max() < 65536
        for ti in range(NT):
            a, b = bounds[ti], bounds[ti + 1]
            n = b - a
            srcg[ti, :n] = es[a:b]
            dstg[ti, :n] = (ed[a:b] - ti * P).astype(np.uint8)
        packed.append({
            "srcp": _pack_cols(srcg.reshape(-1)),          # [128, NT*K] u16
            "cntT": np.ascontiguousarray(cntT.T.astype(np.uint8)),  # [21, NPC]
        })
    return packed, K


def _load_const(nc, pool, dram_ap, shape, dtype):
    sb = pool.tile(shape, dtype, name=f"c_{dram_ap.name}")
    nc.sync.dma_start(out=sb[:], in_=dram_ap[:])
    return sb


def _build(K, TAB):
    nc = bacc.Bacc(None, target_bir_lowering=False)
    RG = [list(range(NCORES))]

    xi = nc.dram_tensor("xi", [P, NT], U16, kind="ExternalInput")
    comb = nc.dram_tensor("comb", [TAB, D], F32, kind="ExternalInput")
    srcp = nc.dram_tensor("srcp", [P, NT * K], U16, kind="ExternalInput")
    cntT = nc.dram_tensor("cntT", [21, NPC], U8, kind="ExternalInput")
    etab = [nc.dram_tensor(f"etab{l}", [21, D], F32, kind="ExternalInput")
            for l in range(2)]
    w1 = [nc.dram_tensor(f"w1_{l}", [D, 2 * D], F32, kind="ExternalInput")
          for l in range(2)]
    w2a = [nc.dram_tensor(f"w2a_{l}", [D, D], F32, kind="ExternalInput")
           for l in range(2)]
    w2b = [nc.dram_tensor(f"w2b_{l}", [D, D], F32, kind="ExternalInput")
           for l in range(2)]
    b1a = [nc.dram_tensor(f"b1a_{l}", [D, 1], F32, kind="ExternalInput")
           for l in range(2)]
    b1b = [nc.dram_tensor(f"b1b_{l}", [D, 1], F32, kind="ExternalInput")
           for l in range(2)]
    b2 = [nc.dram_tensor(f"b2_{l}", [D, 1], F32, kind="ExternalInput")
          for l in range(2)]
    gam = [nc.dram_tensor(f"gam{l}", [D, 1], F32, kind="ExternalInput")
           for l in range(2)]
    bet = [nc.dram_tensor(f"bet{l}", [D, 1], F32, kind="ExternalInput")
           for l in range(2)]
    outr = nc.dram_tensor("outr", [NPC, D], BF16, kind="ExternalOutput")

    from contextlib import ExitStack
    with tile.TileContext(nc) as tc, ExitStack() as ctx:
        const = ctx.enter_context(tc.tile_pool(name="const", bufs=1))
        work = ctx.enter_context(tc.tile_pool(name="work", bufs=4))
        psA = ctx.enter_context(tc.tile_pool(name="psA", bufs=1, space="PSUM"))
        psB = ctx.enter_context(tc.tile_pool(name="psB", bufs=1, space="PSUM"))
        psC = ctx.enter_context(tc.tile_pool(name="psC", bufs=1, space="PSUM"))
        psS = ctx.enter_context(tc.tile_pool(name="psS", bufs=1, space="PSUM"))
        tilec = ctx.enter_context(tc.tile_pool(name="tilec", bufs=1))
        dram = ctx.enter_context(tc.tile_pool(name="dram", bufs=1,
                                              space="DRAM"))

        hloc = [dram.tile([NPC, D], F32, name=f"h{l}loc") for l in range(2)]
        hfull = [dram.tile([N, D], F32, name=f"h{l}full") for l in range(2)]
        st_in = [dram.tile([P, 2], F32, name=f"st_in{l}") for l in range(2)]
        st_out = [dram.tile([P, 2], F32, name=f"st_out{l}") for l in range(2)]

        xi_u = _load_const(nc, const, xi, [P, NT], U16)
        xi_sb = const.tile([P, NT], I32)
        nc.vector.tensor_copy(out=xi_sb[:], in_=xi_u[:])
        srcp_u = _load_const(nc, const, srcp, [P, NT * K], U16)
        srcp_sb = const.tile([P, NT * K], I32)
        nc.vector.tensor_copy(out=srcp_sb[:], in_=srcp_u[:])
        cnt_u = _load_const(nc, const, cntT, [21, NPC], U8)
        cnt_sb = const.tile([21, NPC], F32)
        nc.vector.tensor_copy(out=cnt_sb[:], in_=cnt_u[:])
        iota_i = const.tile([P, P], I32)
        nc.gpsimd.iota(iota_i[:], [[1, P]], channel_multiplier=0)
        iota_sb = const.tile([P, P], F32)
        nc.vector.tensor_copy(out=iota_sb[:], in_=iota_i[:])
        # pcols[e, j] = e + 128*j (edge position within tile, per chunk j)
        pcols_i = const.tile([P, K], I32)
        nc.gpsimd.iota(pcols_i[:], [[P, K]], channel_multiplier=1)
        pcols = const.tile([P, K], F32)
        nc.vector.tensor_copy(out=pcols[:], in_=pcols_i[:])
        rowidx_i = const.tile([P, 1], I32)
        nc.gpsimd.iota(rowidx_i[:], [[1, 1]], channel_multiplier=1)
        rowidx = const.tile([P, 1], F32)
        nc.vector.tensor_copy(out=rowidx[:], in_=rowidx_i[:])
        # UT[k, n] = 1 if k <= n (inclusive-prefix-sum matmul operand)
        UT = const.tile([P, P], F32)
        nc.vector.tensor_tensor(out=UT[:], in0=rowidx[:, :1].to_broadcast([P, P]),
                                in1=iota_sb[:], op=mybir.AluOpType.is_le)
        ones1 = const.tile([1, P], F32)
        nc.vector.memset(ones1[:], 1.0)

        etab_sb = [_load_const(nc, const, etab[l], [21, D], F32)
                   for l in range(2)]
        w1_sb = [_load_const(nc, const, w1[l], [D, 2 * D], F32)
                 for l in range(2)]
        w2a_sb = [_load_const(nc, const, w2a[l], [D, D], F32)
                  for l in range(2)]
        w2b_sb = [_load_const(nc, const, w2b[l], [D, D], F32)
                  for l in range(2)]
        b1a_sb = [_load_const(nc, const, b1a[l], [D, 1], F32)
                  for l in range(2)]
        b1b_sb = [_load_const(nc, const, b1b[l], [D, 1], F32)
                  for l in range(2)]
        b2_sb = [_load_const(nc, const, b2[l], [D, 1], F32) for l in range(2)]
        gam_sb = [_load_const(nc, const, gam[l], [D, 1], F32)
                  for l in range(2)]
        bet_sb = [_load_const(nc, const, bet[l], [D, 1], F32)
                  for l in range(2)]
        ident = const.tile([P, P], F32)
        from concourse.masks import make_identity
        make_identity(nc, ident[:])

        h2all = const.tile([P, NPC], F32)  # reused across both layers

        # ---- h0 for own slice ----
        # indirect-DMA offsets must be static APs: stage the current
        # column into a fixed tile, then gather from it.
        xi_cur = const.tile([P, 1], I32, name="xi_cur")

        def h0_body(ti, rows):
            nc.vector.tensor_copy(out=xi_cur[:], in_=xi_sb[:, bass.ds(ti, 1)])
            hg = work.tile([P, D], F32, name="h0hg")
            nc.gpsimd.indirect_dma_start(
                out=hg[:], out_offset=None, in_=comb[:],
                in_offset=bass.IndirectOffsetOnAxis(ap=xi_cur[:], axis=0))
            nc.sync.dma_start(out=hloc[0][bass.ds(ti * P, rows), :],
                              in_=hg[:rows, :])

        with tc.For_i(0, NFULL) as ti:
            h0_body(ti, P)
        h0_body(NFULL, TAILC)
        nc.gpsimd.collective_compute(
            "AllGather", mybir.AluOpType.bypass, replica_groups=RG,
            ins=[hloc[0].opt()], outs=[hfull[0].opt()])

        src_cur = [const.tile([P, 1], I32, name=f"src_cur{j}")
                   for j in range(K)]

        for l in range(2):
            s_acc = const.tile([P, 2], F32, name=f"s_acc{l}")
            nc.vector.memset(s_acc[:], 0.0)

            def agg_body(ti, cols, l=l):
                # per-tile node-degree prefix sums -> edge-position one-hot
                s1 = psS.tile([P, P], F32, space="PSUM", name="s1")
                cnt_stage = tilec.tile([21, P], F32, name="cnt_stage")
                nc.vector.tensor_copy(out=cnt_stage[:, :cols],
                                      in_=cnt_sb[:, bass.ds(ti * P, cols)])
                nc.tensor.transpose(out=s1[:cols, :21],
                                    in_=cnt_stage[:, :cols],
                                    identity=ident[:21, :21])
                cnt21 = tilec.tile([P, 21], F32, name="cnt21")
                nc.vector.tensor_copy(out=cnt21[:cols, :], in_=s1[:cols, :21])
                degP = tilec.tile([P, 1], F32, name="degP")
                nc.vector.memset(degP[:], 0.0)
                nc.vector.reduce_sum(out=degP[:cols], in_=cnt21[:cols, :21],
                                     axis=mybir.AxisListType.X)
                cumI = tilec.tile([P, 1], F32, name="cumI")
                nc.tensor.matmul(out=s1[:, 0:1], lhsT=UT[:], rhs=degP[:],
                                 start=True, stop=True, skip_group_check=True)
                nc.vector.tensor_copy(out=cumI[:], in_=s1[:, 0:1])
                cumE = tilec.tile([P, 1], F32, name="cumE")
                nc.vector.tensor_tensor(out=cumE[:], in0=cumI[:], in1=degP[:],
                                        op=mybir.AluOpType.subtract)
                cumIrep = tilec.tile([P, P], F32, name="cumIrep")
                cumErep = tilec.tile([P, P], F32, name="cumErep")
                for src_c, rep in ((cumI, cumIrep), (cumE, cumErep)):
                    nc.tensor.transpose(out=s1[:1, :], in_=src_c[:, :1],
                                        identity=ident[:])
                    row = tilec.tile([1, P], F32, name=f"row_{rep.tensor.name}")
                    nc.vector.tensor_copy(out=row[:1, :], in_=s1[:1, :])
                    nc.tensor.matmul(out=s1[:, :cols], lhsT=ones1[:1, :],
                                     rhs=row[:1, :cols], start=True, stop=True,
                                     skip_group_check=True)
                    nc.vector.tensor_copy(out=rep[:, :cols], in_=s1[:, :cols])

                agg_ps = psA.tile([P, P], F32, space="PSUM", name="agg_ps")
                nc.tensor.matmul(
                    out=agg_ps[:, :cols], lhsT=etab_sb[l][:],
                    rhs=cnt_sb[:, bass.ds(ti * P, cols)],
                    start=True, stop=False, skip_group_check=True)
                for j in range(K):
                    col = ti * K + j
                    nc.vector.tensor_copy(out=src_cur[j][:],
                                          in_=srcp_sb[:, bass.ds(col, 1)])
                    hg = work.tile([P, D], F32, name="ehg")
                    nc.gpsimd.indirect_dma_start(
                        out=hg[:], out_offset=None, in_=hfull[l][:],
                        in_offset=bass.IndirectOffsetOnAxis(
                            ap=src_cur[j][:], axis=0))
                    ge = work.tile([P, P], F32, name="ege")
                    nc.vector.tensor_tensor(
                        out=ge[:, :cols],
                        in0=pcols[:, j:j + 1].to_broadcast([P, cols]),
                        in1=cumErep[:, :cols], op=mybir.AluOpType.is_ge)
                    oh = work.tile([P, P], F32, name="eoh")
                    nc.vector.tensor_tensor(
                        out=oh[:, :cols],
                        in0=pcols[:, j:j + 1].to_broadcast([P, cols]),
                        in1=cumIrep[:, :cols], op=mybir.AluOpType.is_lt)
                    nc.vector.tensor_tensor(out=oh[:, :cols], in0=oh[:, :cols],
                                            in1=ge[:, :cols],
                                            op=mybir.AluOpType.mult)
                    nc.tensor.matmul(
                        out=agg_ps[:, :cols], lhsT=hg[:], rhs=oh[:, :cols],
                        start=False, stop=(j == K - 1), skip_group_check=True)
                aggT = work.tile([P, P], F32, name="aggT")
                nc.vector.tensor_copy(out=aggT[:, :cols], in_=agg_ps[:, :cols])

                r = []
                for half, bsb in ((0, b1a_sb[l]), (1, b1b_sb[l])):
                    z_ps = psB.tile([P, P], F32, space="PSUM", name=f"z{half}")
                    nc.tensor.matmul(
                        out=z_ps[:, :cols],
                        lhsT=w1_sb[l][:, half * D:(half + 1) * D],
                        rhs=aggT[:, :cols], start=True, stop=True,
                        skip_group_check=True)
                    rh = work.tile([P, P], F32, name=f"rh{half}")
                    nc.vector.tensor_tensor(
                        out=rh[:, :cols], in0=z_ps[:, :cols],
                        in1=bsb[:, :1].to_broadcast([P, cols]),
                        op=mybir.AluOpType.add)
                    nc.vector.tensor_scalar_max(rh[:, :cols], rh[:, :cols],
                                                0.0)
                    r.append(rh)
                h2_ps = psC.tile([P, P], F32, space="PSUM", name="h2ps")
                nc.tensor.matmul(out=h2_ps[:, :cols], lhsT=w2a_sb[l][:],
                                 rhs=r[0][:, :cols], start=True, stop=False,
                                 skip_group_check=True)
                nc.tensor.matmul(out=h2_ps[:, :cols], lhsT=w2b_sb[l][:],
                                 rhs=r[1][:, :cols], start=False, stop=True,
                                 skip_group_check=True)
                dst = h2all[:, bass.ds(ti * P, cols)]
                nc.vector.tensor_tensor(
                    out=dst, in0=h2_ps[:, :cols],
                    in1=b2_sb[l][:, :1].to_broadcast([P, cols]),
                    op=mybir.AluOpType.add)
                part = work.tile([P, 1], F32, name="part")
                nc.vector.reduce_sum(out=part[:], in_=dst,
                                     axis=mybir.AxisListType.X)
                nc.vector.tensor_add(s_acc[:, 0:1], s_acc[:, 0:1], part[:])
                sq = work.tile([P, P], F32, name="sq")
                nc.vector.tensor_mul(sq[:, :cols], dst, dst)
                part2 = work.tile([P, 1], F32, name="part2")
                nc.vector.reduce_sum(out=part2[:], in_=sq[:, :cols],
                                     axis=mybir.AxisListType.X)
                nc.vector.tensor_add(s_acc[:, 1:2], s_acc[:, 1:2], part2[:])

            with tc.For_i(0, NFULL) as ti:
                agg_body(ti, P)
            agg_body(NFULL, TAILC)

            # ---- AllReduce BN stats ----
            nc.sync.dma_start(out=st_in[l][:], in_=s_acc[:])
            nc.gpsimd.collective_compute(
                "AllReduce", mybir.AluOpType.add, replica_groups=RG,
                ins=[st_in[l].opt()], outs=[st_out[l].opt()])
            st_sb = const.tile([P, 2], F32, name=f"st_sb{l}")
            nc.sync.dma_start(out=st_sb[:], in_=st_out[l][:])

            # BN coefficients: a = gamma*rsqrt(var+eps), b = beta - a*mu
            mu = const.tile([P, 1], F32, name=f"mu{l}")
            nc.vector.tensor_scalar_mul(mu[:], st_sb[:, 0:1], 1.0 / N)
            ex2 = const.tile([P, 1], F32, name=f"ex2{l}")
            nc.vector.tensor_scalar_mul(ex2[:], st_sb[:, 1:2], 1.0 / N)
            var = const.tile([P, 1], F32, name=f"var{l}")
            nc.vector.tensor_mul(var[:], mu[:], mu[:])
            nc.vector.tensor_tensor(out=var[:], in0=ex2[:], in1=var[:],
                                    op=mybir.AluOpType.subtract)
            nc.vector.tensor_scalar_add(var[:], var[:], BN_EPS)
            std = const.tile([P, 1], F32, name=f"std{l}")
            nc.scalar.activation(out=std[:], in_=var[:],
                                 func=mybir.ActivationFunctionType.Sqrt)
            rstd = const.tile([P, 1], F32, name=f"rstd{l}")
            nc.vector.reciprocal(out=rstd[:], in_=std[:])
            a = const.tile([P, 1], F32, name=f"a{l}")
            nc.vector.tensor_mul(a[:], gam_sb[l][:], rstd[:])
            b = const.tile([P, 1], F32, name=f"b{l}")
            nc.vector.tensor_mul(b[:], a[:], mu[:])
            nc.vector.tensor_tensor(out=b[:], in0=bet_sb[l][:], in1=b[:],
                                    op=mybir.AluOpType.subtract)

            # ---- BN apply (+relu l=0), transpose, write rows ----
            def bn_body(ti, cols, l=l, a=a, b=b):
                src = h2all[:, bass.ds(ti * P, cols)]
                xt = work.tile([P, P], F32, name="bnxt")
                nc.vector.tensor_tensor(
                    out=xt[:, :cols], in0=src,
                    in1=a[:, :1].to_broadcast([P, cols]),
                    op=mybir.AluOpType.mult)
                nc.vector.tensor_tensor(
                    out=xt[:, :cols], in0=xt[:, :cols],
                    in1=b[:, :1].to_broadcast([P, cols]),
                    op=mybir.AluOpType.add)
                if l == 0:
                    nc.vector.tensor_scalar_max(xt[:, :cols], xt[:, :cols],
                                                0.0)
                tp = psC.tile([P, P], F32, space="PSUM", name="bntp")
                nc.tensor.transpose(out=tp[:cols, :], in_=xt[:, :cols],
                                    identity=ident[:])
                if l == 0:
                    hrow = work.tile([P, D], F32, name="bnrow0")
                    nc.vector.tensor_copy(out=hrow[:cols, :], in_=tp[:cols, :])
                    nc.sync.dma_start(out=hloc[1][bass.ds(ti * P, cols), :],
                                      in_=hrow[:cols, :])
                else:
                    hrow = work.tile([P, D], BF16, name="bnrow1")
                    nc.vector.tensor_copy(out=hrow[:cols, :], in_=tp[:cols, :])
                    nc.sync.dma_start(out=outr[bass.ds(ti * P, cols), :],
                                      in_=hrow[:cols, :])

            with tc.For_i(0, NFULL) as ti:
                bn_body(ti, P)
            bn_body(NFULL, TAILC)

            if l == 0:
                nc.gpsimd.collective_compute(
                    "AllGather", mybir.AluOpType.bypass, replica_groups=RG,
                    ins=[hloc[1].opt()], outs=[hfull[1].opt()])
    nc.compile()
    return nc


LAUNCH_NS = []


def _run(nc, maps, cores):
    import time as _t
    t0 = _t.monotonic_ns()
    res = run_bass_kernel_spmd(nc, maps, cores)
    dt = _t.monotonic_ns() - t0
    LAUNCH_NS.append(res.exec_time_ns if res.exec_time_ns else dt)
    return res


def kernel(x, edge_index, edge_attr, batch, xemb1, xemb2, e1, e2,
           W1, b1, W2, b2, gamma, beta):
    LAUNCH_NS.clear()
    f32 = np.float32
    packed, K = _host_prep(x, edge_index, edge_attr)

    x = np.asarray(x)
    nu0 = int(x[:, 0].max()) + 1
    nu1 = int(x[:, 1].max()) + 1
    xi_full = (x[:, 0] * nu1 + x[:, 1]).astype(np.uint16)
    TAB = max(16, nu0 * nu1)
    comb = np.zeros((TAB, D), f32)
    comb[:nu0 * nu1] = (np.asarray(xemb1, f32)[:nu0, None, :]
                        + np.asarray(xemb2, f32)[None, :nu1, :]
                        ).reshape(nu0 * nu1, D)

    shared = {"comb": comb}
    for l in range(2):
        shared[f"etab{l}"] = (
            np.repeat(np.asarray(e1[l], f32), 3, axis=0)
            + np.tile(np.asarray(e2[l], f32), (7, 1))).copy()
        shared[f"w1_{l}"] = np.asarray(W1[l], f32).copy()
        shared[f"w2a_{l}"] = np.asarray(W2[l][:D], f32).copy()
        shared[f"w2b_{l}"] = np.asarray(W2[l][D:], f32).copy()
        shared[f"b1a_{l}"] = np.asarray(b1[l][:D], f32).reshape(D, 1).copy()
        shared[f"b1b_{l}"] = np.asarray(b1[l][D:], f32).reshape(D, 1).copy()
        shared[f"b2_{l}"] = np.asarray(b2[l], f32).reshape(D, 1).copy()
        shared[f"gam{l}"] = np.asarray(gamma[l], f32).reshape(D, 1).copy()
        shared[f"bet{l}"] = np.asarray(beta[l], f32).reshape(D, 1).copy()

    cores = list(range(NCORES))
    nc = _build(K, TAB)
    maps = []
    for c in cores:
        xi_pad = np.zeros(NT * P, np.uint16)
        xi_pad[:NPC] = xi_full[c * NPC:(c + 1) * NPC]
        m = {"xi": _pack_cols(xi_pad),
             "srcp": packed[c]["srcp"],
             "cntT": packed[c]["cntT"]}
        m.update(shared)
        maps.append(m)
    res = _run(nc, maps, cores).results
    out = np.concatenate([r["outr"] for r in res], axis=0)
    return out.astype(np.float32)
